# revision 1
# baseline (speedup 1.0000x reference)
"""nn_Conductor: 2-layer LSTM (H=1024, T=32, B=512) on 8 TRN2 NeuronCores.

Strategy (weight-stationary tensor parallelism, NOT the data-parallel hint):
the hidden dim H=1024 is sharded 8 ways. Core k owns hidden units
[k*128,(k+1)*128) of BOTH layers and keeps the corresponding lhsT slices of
W_ih/W_hh (all 4 gates, both layers, bf16) resident in SBUF for the whole
kernel — so the 67MB of weights are read from HBM exactly once instead of
once per timestep (the memory-bound regime the naive data-parallel split
lands in). Each core computes its gate slice for the FULL batch (matmul
free dim N=512), updates its slice of (h, c), and AllGathers the bf16
h-slice [128, 512] after each layer so every core holds the full h^T
[1024, 512] needed as the next matmul's rhs.

Per step, per layer, per core: 64 matmuls [K=128, M=128, N=512] (bf16,
fp32 PSUM accumulation), 5 activations + 4 DVE elementwise ops, one
128KB SBUF->HBM bounce, one 8-rank AllGather, one coalesced 1MB
HBM->SBUF gather (single InstDMACopy so it fans across all 16 SDMA
engines). The h-part matmuls (whose rhs is last step's h) overlap the
collective; the x-part runs m-tile-major so gate activations overlap the
matmul stream.

The h0 = tanh(z @ to_h_W.T + b) init is computed replicated on every core
(128 matmuls, one-off). State c stays fp32; matmul operands are bf16
(measured end-to-end rel err ~4e-3 vs the fp32 reference).
"""
import sys

sys.path.insert(0, "/opt/trn_rl_repo")

import numpy as np
from ml_dtypes import bfloat16

import concourse.bacc as bacc
import concourse.mybir as mybir
import concourse.tile as tile

H, L, T, B = 1024, 2, 32, 512
P = 128
NC = 8
KT = H // P
S = H // NC
F32 = mybir.dt.float32
BF16 = mybir.dt.bfloat16
Sig = mybir.ActivationFunctionType.Sigmoid
Tanh = mybir.ActivationFunctionType.Tanh


def _prep_inputs(z, to_h_W, to_h_b, init_emb, W_ih, W_hh, b_ih, b_hh):
    z = np.asarray(z, np.float32)
    to_h_W = np.asarray(to_h_W, np.float32)
    to_h_b = np.asarray(to_h_b, np.float32)
    init_emb = np.asarray(init_emb, np.float32)
    W_ih = np.asarray(W_ih, np.float32)
    W_hh = np.asarray(W_hh, np.float32)
    b_comb = np.asarray(b_ih, np.float32) + np.asarray(b_hh, np.float32)

    zT = np.ascontiguousarray(z.T).astype(bfloat16)
    x0T = np.broadcast_to(init_emb[0][:, None], (H, B)).astype(bfloat16)
    tohT = np.ascontiguousarray(to_h_W.T)
    toh_blocks = np.empty((2 * KT, KT, P, P), np.float32)
    for mi in range(2 * KT):
        for kt in range(KT):
            toh_blocks[mi, kt] = tohT[kt * P:(kt + 1) * P, mi * P:(mi + 1) * P]
    toh_arr = toh_blocks.reshape(2 * KT * KT * P, P).astype(bfloat16)
    tohb = np.ascontiguousarray(to_h_b.reshape(2 * KT, P).T).astype(np.float32)

    in_maps = []
    for k in range(NC):
        w_blocks = np.empty((4, KT, P, 4 * P), np.float32)
        for mat_i, Wfull in enumerate((W_ih[0], W_hh[0], W_ih[1], W_hh[1])):
            rows = np.concatenate(
                [Wfull[q * H + k * S: q * H + k * S + P, :] for q in range(4)],
                axis=0)
            WT = np.ascontiguousarray(rows.T)
            for kt in range(KT):
                w_blocks[mat_i, kt] = WT[kt * P:(kt + 1) * P, :]
        w_arr = w_blocks.reshape(4 * KT * P, 4 * P).astype(bfloat16)
        b_arr = np.empty((P, 8), np.float32)
        for l in range(L):
            for q in range(4):
                b_arr[:, l * 4 + q] = b_comb[l, q * H + k * S: q * H + k * S + P]
        in_maps.append({"w": w_arr, "b": b_arr, "toh": toh_arr, "tohb": tohb,
                        "zT": zT, "x0T": x0T})
    return in_maps


def _build():
    nc = bacc.Bacc("TRN2", target_bir_lowering=False, debug=False)

    w_ext = nc.declare_dram_parameter("w", [4 * KT * P, 4 * P], BF16, isOutput=False)
    b_ext = nc.declare_dram_parameter("b", [P, 8], F32, isOutput=False)
    toh_ext = nc.declare_dram_parameter("toh", [2 * KT * KT * P, P], BF16, isOutput=False)
    tohb_ext = nc.declare_dram_parameter("tohb", [P, 2 * KT], F32, isOutput=False)
    zT_ext = nc.declare_dram_parameter("zT", [H, B], BF16, isOutput=False)
    x0T_ext = nc.declare_dram_parameter("x0T", [H, B], BF16, isOutput=False)
    ys_ext = nc.declare_dram_parameter("ys", [T * P, B], F32, isOutput=True)

    with tile.TileContext(nc) as tc:
        with (
            tc.tile_pool(name="const", bufs=1) as const,
            tc.tile_pool(name="state", bufs=1) as state,
            tc.tile_pool(name="hT", bufs=6) as hTp,
            tc.tile_pool(name="act", bufs=2) as actp,
            tc.tile_pool(name="psum", bufs=8, space="PSUM") as psum,
            tc.tile_pool(name="dram", bufs=3, space="DRAM") as dram,
        ):
            w_sb = const.tile([P, 4 * KT * 4 * P], BF16)
            for g in range(4 * KT):
                nc.sync.dma_start(w_sb[:, g * 512:(g + 1) * 512],
                                  w_ext[g * P:(g + 1) * P, :])
            b_sb = const.tile([P, 8], F32)
            nc.sync.dma_start(b_sb[:], b_ext[:])
            toh_sb = const.tile([P, 2 * KT * KT * P], BF16)
            for g in range(2 * KT * KT):
                nc.sync.dma_start(toh_sb[:, g * P:(g + 1) * P],
                                  toh_ext[g * P:(g + 1) * P, :])
            tohb_sb = const.tile([P, 2 * KT], F32)
            nc.sync.dma_start(tohb_sb[:], tohb_ext[:])
            zT_sb = const.tile([P, KT * B], BF16)
            for kt in range(KT):
                nc.sync.dma_start(zT_sb[:, kt * B:(kt + 1) * B],
                                  zT_ext[kt * P:(kt + 1) * P, :])
            x0T_sb = const.tile([P, KT * B], BF16)
            for kt in range(KT):
                nc.sync.dma_start(x0T_sb[:, kt * B:(kt + 1) * B],
                                  x0T_ext[kt * P:(kt + 1) * P, :])

            c_sb = [state.tile([P, B], F32, tag=f"c{l}", name=f"c{l}") for l in range(L)]
            for l in range(L):
                nc.any.memset(c_sb[l][:], 0.0)

            h_init = [hTp.tile([P, KT * B], BF16, tag="hT", name=f"h_init{l}")
                      for l in range(L)]
            for mi in range(2 * KT):
                ps = psum.tile([P, B], F32, tag="gates", name=f"ig{mi}")
                for kt in range(KT):
                    g = mi * KT + kt
                    nc.tensor.matmul(ps[:],
                                     toh_sb[:, g * P:(g + 1) * P],
                                     zT_sb[:, kt * B:(kt + 1) * B],
                                     start=(kt == 0), stop=(kt == KT - 1))
                l, kh = divmod(mi, KT)
                nc.scalar.activation(h_init[l][:, kh * B:(kh + 1) * B], ps[:],
                                     Tanh, bias=tohb_sb[:, mi:mi + 1])

            def layer_step(l, x_rhs, h_rhs, want_f32):
                ih, hh = 2 * l, 2 * l + 1
                ps = [psum.tile([P, B], F32, tag="gates", name=f"gates{m}")
                      for m in range(4)]
                # h-part: rhs ready since last step; overlaps the in-flight AG
                for kt in range(KT):
                    g = hh * KT + kt
                    for m in range(4):
                        nc.tensor.matmul(
                            ps[m][:],
                            w_sb[:, g * 512 + m * P: g * 512 + (m + 1) * P],
                            h_rhs[:, kt * B:(kt + 1) * B],
                            start=(kt == 0), stop=False)
                # x-part m-outer so each gate's PSUM group closes early and its
                # activation overlaps the remaining matmuls
                for m in range(4):
                    for kt in range(KT):
                        g = ih * KT + kt
                        nc.tensor.matmul(
                            ps[m][:],
                            w_sb[:, g * 512 + m * P: g * 512 + (m + 1) * P],
                            x_rhs[:, kt * B:(kt + 1) * B],
                            start=False, stop=(kt == KT - 1))
                i_t = actp.tile([P, B], F32, tag="i")
                f_t = actp.tile([P, B], F32, tag="f")
                g_t = actp.tile([P, B], F32, tag="g")
                o_t = actp.tile([P, B], F32, tag="o")
                nc.scalar.activation(i_t[:], ps[0][:], Sig, bias=b_sb[:, 4 * l + 0:4 * l + 1])
                nc.scalar.activation(f_t[:], ps[1][:], Sig, bias=b_sb[:, 4 * l + 1:4 * l + 2])
                nc.scalar.activation(g_t[:], ps[2][:], Tanh, bias=b_sb[:, 4 * l + 2:4 * l + 3])
                nc.scalar.activation(o_t[:], ps[3][:], Sig, bias=b_sb[:, 4 * l + 3:4 * l + 4])
                t1 = actp.tile([P, B], F32, tag="t1")
                t2 = actp.tile([P, B], F32, tag="t2")
                nc.vector.tensor_mul(t1[:], f_t[:], c_sb[l][:])
                nc.vector.tensor_mul(t2[:], i_t[:], g_t[:])
                nc.vector.tensor_add(c_sb[l][:], t1[:], t2[:])
                tc_t = actp.tile([P, B], F32, tag="tc")
                nc.scalar.activation(tc_t[:], c_sb[l][:], Tanh)
                h_f32 = None
                h_bf = actp.tile([P, B], BF16, tag="hbf")
                nc.vector.tensor_mul(h_bf[:], o_t[:], tc_t[:])
                if want_f32:
                    # independent second mul: ys output path off the chain
                    h_f32 = actp.tile([P, B], F32, tag="hf32")
                    nc.vector.tensor_mul(h_f32[:], o_t[:], tc_t[:])
                ag_in = dram.tile([P, B], BF16, tag="agin")
                nc.sync.dma_start(ag_in[:], h_bf[:])
                hT_new = hTp.tile([P, KT * B], BF16, tag="hT")
                ag_out = dram.tile([H, B], BF16, tag="agout", addr_space="Shared")
                nc.gpsimd.collective_compute(
                    "AllGather", mybir.AluOpType.bypass,
                    ins=[ag_in.opt()], outs=[ag_out.opt()],
                    replica_groups=[list(range(NC))])
                # one coalesced gather: a single InstDMACopy fans across all
                # 16 SDMA engines; 8 separate ones serialize on the HWDGE ring
                nc.sync.dma_start(
                    hT_new[:].rearrange("p (t n) -> p t n", t=KT),
                    ag_out[:].rearrange("(t p) n -> p t n", p=P))
                return hT_new, h_f32

            x_rhs = x0T_sb
            h_prev = h_init
            for t in range(T):
                h0T, _ = layer_step(0, x_rhs, h_prev[0], want_f32=False)
                h1T, h1_f32 = layer_step(1, h0T, h_prev[1], want_f32=True)
                nc.gpsimd.dma_start(ys_ext[t * P:(t + 1) * P, :], h1_f32[:])
                x_rhs = h1T
                h_prev = [h0T, h1T]

    nc.compile()
    return nc


_CACHE = {}


def kernel(**inputs) -> np.ndarray:
    if "nc" not in _CACHE:
        _CACHE["nc"] = _build()
    nc = _CACHE["nc"]
    in_maps = _prep_inputs(**inputs)

    from concourse.bass_utils import run_bass_kernel_spmd
    res = run_bass_kernel_spmd(nc, in_maps, list(range(NC)))
    slabs = [r["ys"].reshape(T, P, B).transpose(0, 2, 1) for r in res.results]
    return np.ascontiguousarray(np.concatenate(slabs, axis=2)).astype(np.float32)

